# revision 32
# baseline (speedup 1.0000x reference)
"""Trainium2 Bass kernel for 7x7 local (sparse) attention, SPMD over 8 NeuronCores.

Math (per channel c, pixel p):
    q = w_q @ x, k = w_k @ x            (1x1 convs)
    logit[c,p,(i,j)] = q[c,p] * (kpad[c, p+(i,j)] + rel[c,(i,j)])
    out[c,p] = sum_k softmax_k(logit) * vpad[c, p+(i,j)]
where rel[c,(i,j)] = rel_h[c,i] for c<64, rel_w[c-64,j] for c>=64, and
kpad/vpad are zero-padded by 3 (padded taps contribute exp(q*rel) to the
softmax denominator -- matching the reference exactly).

Sharding: 8 cores = 2 batches x 2 CHANNEL halves x 2 spatial 48-line tiles.
High-half cores store all spatial data TRANSPOSED (W-major), so that the rel
bias always attaches to the stored-ROW tap (rel_h[c,i] for low cores, rel_w
[c,j] for high ones) and one NEFF serves all 8 cores.  Within a core the
128 partitions hold (64 channels x 2 row-halves of 24 lines each).

This layout lets the per-tap "krel = k-window + rel" pass (49 taps worth of
adds per pixel) collapse into SEVEN pre-biased k planes built once per core
(plane_m = k + rel[c,m]; window overlap gives the 7x redundancy win), plus a
1-col-shifted B copy of each for the odd column taps (DVE 2x mode needs
4B-aligned windows).  The logit tensor_tensor then reads plane windows
directly:  logit[c,(dR,dC),p] = q[c,p] * plane_dR[c, p + (dR,dC)].

Per-core device pipeline (21 groups = 3 row-epochs x 7 dC):
  DVE: one batched tensor_tensor per group for logits (q broadcast over the
       7 dR via a stride-0 AP reading the 7 planes via a strided window AP)
       and one for e*v (overlapping-row v AP), both fp16/bf16 2x mode.
  ACT: one exp per group (7 dR batched), and the final recip = exp(-ln(den)).
  PE:  q/k 1x1 convs (64-wide weights, written into both partition halves),
       then one identity-matmul per (dR, 256-px chunk) accumulating
       [num | den] jointly into 5 PSUM banks per epoch.
The emission is software-pipelined by one group so ScalarE's exp[g] overlaps
group g+1's logit TT.  Output epoch = 8 lines x 96 cols: [num|den] for 768 px
= 3 PSUM banks, so two epochs' accumulators coexist and the epoch boundary
(normalize of e vs first matmuls of e+1) fully overlaps.

fp16 is used for the whole logit chain (q, planes, logit): bf16's 8-bit
mantissa on |logit|<=60 costs ~4% absmax error; fp16 keeps it ~1%.
e/prod are bf16 (need range: e up to exp(60)).
"""

import sys

import numpy as np
import ml_dtypes

sys.path.insert(0, "/opt/trn_rl_repo")

import concourse.bass as bass  # noqa: E402
import concourse.tile as tile  # noqa: E402
from concourse import mybir  # noqa: E402
from concourse.vector_clock import ScopedClock  # noqa: E402

F32 = mybir.dt.float32
BF16 = mybir.dt.bfloat16
FP16 = mybir.dt.float16

B, CIN, COUT, H, W, K, PAD = 2, 128, 128, 96, 96, 7, 3
NCORES = 8
RT = 48          # stored-R lines per core
RH = 24          # lines per partition half
SLAB = RH + 2 * PAD   # 30 rows incl halo per half
XSLAB = RT + 2 * PAD  # 54 rows of x slab
PW = 102         # padded plane width (3 + 96 + 3)
WPAD = 104       # v buffer width (96 + 6 pad + 2 spare)
EP_ROWS = 8      # lines per PSUM epoch
EP = RH // EP_ROWS    # 3 epochs
FD = EP_ROWS * W      # 768
PLSTRIDE = SLAB * PW  # 3060 elements per plane
# PSUM chunking: each bank holds [num(256 px) | den(same px)] so one matmul
# accumulates both. 768 px = 3 chunks of 256 -> 3 banks, 3 MMs; two epochs'
# accumulators coexist (3+3 of 8 banks) so the epoch boundary overlaps.
CHUNKS = [(0, 256), (256, 256), (512, 256)]

_cache = {}


def _patch_tile_drain():
    """walrus in this container allows only one sync-wait per instruction;
    split excess waits onto NoOps."""
    if getattr(tile, "_drain_patched", False):
        return

    def _drain_and_barrier(self, tick_clock, wait_clock):
        drain_inst = self.nc.sync.drain()
        wait_clock.add_sem_waits(
            drain_inst.ins, ScopedClock({None: tick_clock.global_clock})
        )
        si = drain_inst.ins.sync_info
        if si is not None and si.on_wait and len(si.on_wait) > 1:
            waits = list(si.on_wait)
            drain_inst.ins.sync_info = mybir.SyncInfo(
                on_wait=waits[:1], on_update=list(si.on_update)
            )
            for w in waits[1:]:
                nop_inst = self.nc.sync.nop()
                nop_inst.ins.sync_info = mybir.SyncInfo(on_wait=[w], on_update=[])
        self.nc.all_engine_barrier()
        assert self.sems is not None
        popped = self.nc._tile_sem_poison_stack.pop()
        assert popped is self._sem_poison
        self.nc.clear_and_free_semaphores(list(self.sems.allocated().values()))
        self.nc.all_engine_barrier()

    tile.TileContext._drain_and_barrier = _drain_and_barrier
    tile._drain_patched = True


_split_ctr = [0]


def _split_sync_waits(nc, maxw=1):
    for fn in nc.m.functions:
        for bb in fn.blocks:
            if not any(
                inst.sync_info is not None
                and inst.sync_info.on_wait
                and len(inst.sync_info.on_wait) > maxw
                for inst in bb.instructions
            ):
                continue
            new_list = []
            for inst in bb.instructions:
                si = inst.sync_info
                if si is not None and si.on_wait and len(si.on_wait) > maxw:
                    waits = list(si.on_wait)
                    extra, keep = waits[:-maxw], waits[-maxw:]
                    for i in range(0, len(extra), maxw):
                        _split_ctr[0] += 1
                        nop = mybir.InstNoOp(
                            name=f"splitw-{_split_ctr[0]}", ins=[], outs=[]
                        )
                        nop.engine = inst.engine
                        nop.sync_info = mybir.SyncInfo(
                            on_wait=extra[i : i + maxw], on_update=[]
                        )
                        new_list.append(nop)
                    inst.sync_info = mybir.SyncInfo(
                        on_wait=keep, on_update=list(si.on_update)
                    )
                new_list.append(inst)
            try:
                bb.instructions = new_list
            except Exception:
                bb.instructions.clear()
                bb.instructions.extend(new_list)


def _build():
    _patch_tile_drain()
    nc = bass.Bass("TRN2", target_bir_lowering=False, debug=False)

    xs_ext = nc.dram_tensor("xs", [128, XSLAB * W], FP16, kind="ExternalInput").ap()
    va_ext = nc.dram_tensor("vbufA", [128, SLAB, WPAD], BF16, kind="ExternalInput").ap()
    vb_ext = nc.dram_tensor("vbufB", [128, SLAB, WPAD], BF16, kind="ExternalInput").ap()
    wq_ext = nc.dram_tensor("wqT", [128, 64], FP16, kind="ExternalInput").ap()
    wk_ext = nc.dram_tensor("wkT", [128, 64], FP16, kind="ExternalInput").ap()
    rel_ext = nc.dram_tensor("relcols", [128, 8], F32, kind="ExternalInput").ap()
    id_ext = nc.dram_tensor("ident", [128, 128], BF16, kind="ExternalInput").ap()
    out_ext = nc.dram_tensor("out", [128, RH * W], F32, kind="ExternalOutput").ap()

    from contextlib import ExitStack

    with tile.TileContext(nc) as tc, ExitStack() as ctx:
        consts = ctx.enter_context(tc.tile_pool(name="consts", bufs=1))
        main = ctx.enter_context(tc.tile_pool(name="main", bufs=1))

        wqT = consts.tile([128, 64], FP16)
        wkT = consts.tile([128, 64], FP16)
        ident = consts.tile([128, 128], BF16)
        relc = consts.tile([128, 8], F32)
        vbufA = main.tile([128, SLAB, WPAD], BF16)
        vbufB = main.tile([128, SLAB, WPAD], BF16)
        q_sbs = [main.tile([128, FD], FP16, name=f"q{e}") for e in range(EP)]
        planesA = main.tile([128, K, SLAB, PW], FP16)
        planesB = main.tile([128, K, SLAB, PW], FP16)

        # ---- preamble: q/k 1x1 convs into the (channel, row-half) layout
        with tc.tile_pool(name="xsp", bufs=1) as xsp:
            xs = xsp.tile([128, XSLAB * W], FP16)
            # xs alone on the sync queue (it gates the k matmuls); weights +
            # bulk v + consts on scalar so they land in parallel.
            nc.sync.dma_start(out=xs[:, 0 : SLAB * W], in_=xs_ext[:, 0 : SLAB * W])
            nc.sync.dma_start(
                out=xs[:, SLAB * W : XSLAB * W], in_=xs_ext[:, SLAB * W : XSLAB * W]
            )
            nc.scalar.dma_start(out=wkT, in_=wk_ext)
            nc.scalar.dma_start(out=wqT, in_=wq_ext)
            nc.scalar.dma_start(out=relc, in_=rel_ext)
            nc.scalar.dma_start(out=ident, in_=id_ext)
            nc.scalar.dma_start(out=vbufA, in_=va_ext)
            nc.scalar.dma_start(out=vbufB, in_=vb_ext)
            # dummy first ACTIVATE: hoists the ~1.3us ACT_TABLE_LOAD off the
            # k_conv -> plane0 critical path (it attaches to the first ACT op)
            scr = consts.tile([128, 1], F32)
            nc.scalar.activation(
                out=scr, in_=relc[:, 0:1],
                func=mybir.ActivationFunctionType.Exp, bias=0.0, scale=1.0,
            )

            NPRE = 6
            KW = SLAB * W // NPRE  # 480 = 5 rows
            QW = RH * W // NPRE    # 384 = 4 rows
            with tc.tile_pool(name="ps_pre", bufs=1, space="PSUM") as ps_pre:
                # k first: it gates the plane build (q only gates logits)
                k_ps = ps_pre.tile([128, NPRE, 512], F32, tag="pre")
                # chunk-major: plane0A's first half reads chunks 0-2 of BOTH
                # partition halves, so emit those 6 matmuls first
                for c in range(NPRE):
                    for rh in range(2):
                        base = RH * rh * W
                        nc.tensor.matmul(
                            k_ps[64 * rh : 64 * rh + 64, c, 0:KW],
                            wkT,
                            xs[:, base + c * KW : base + (c + 1) * KW],
                            start=True, stop=True,
                        )
                # plane 0 (A and B copies) straight from PSUM with the m=0
                # rel bias; chunk c covers slab rows 5c..5c+4
                k_src = k_ps[:, :, 0:KW].rearrange("p c (r w) -> p c r w", r=5)
                p0A = planesA[:, 0, :, 3 : 3 + W].rearrange(
                    "p (c r) w -> p c r w", c=NPRE
                )
                p0B = planesB[:, 0, :, 2 : 2 + W].rearrange(
                    "p (c r) w -> p c r w", c=NPRE
                )
                # plane0A in two row-halves: the first logit only reads plane
                # rows 0..13, so it can start before the bottom half lands.
                nc.scalar.activation(
                    out=p0A[:, 0:3], in_=k_src[:, 0:3],
                    func=mybir.ActivationFunctionType.Identity,
                    bias=relc[:, 0:1], scale=1.0,
                )
                nc.scalar.activation(
                    out=p0A[:, 3:NPRE], in_=k_src[:, 3:NPRE],
                    func=mybir.ActivationFunctionType.Identity,
                    bias=relc[:, 0:1], scale=1.0,
                )
                # q gets its OWN 2 PSUM banks (6 k + 2 q = 8): reusing the k
                # banks would stall the q convs behind the plane0 ACT reads.
                # Both partition halves share a bank; one wave per epoch.
                q_ps = ps_pre.tile([128, 2, 512], F32, tag="qps")
                for w in range(EP):
                    for j in range(2):
                        for rh in range(2):
                            base = (RH * rh + PAD) * W + (2 * w + j) * QW
                            nc.tensor.matmul(
                                q_ps[64 * rh : 64 * rh + 64, j, 0:QW],
                                wqT,
                                xs[:, base : base + QW],
                                start=True, stop=True,
                            )
                    nc.scalar.copy(
                        out=q_sbs[w].rearrange("p (c w) -> p c w", c=2),
                        in_=q_ps[:, :, 0:QW],
                    )
                    if w == 0:
                        nc.scalar.activation(
                            out=p0B, in_=k_src,
                            func=mybir.ActivationFunctionType.Identity,
                            bias=relc[:, 0:1], scale=1.0,
                        )

        def strips(planes, ranges):
            """pad strips for plane 0: value = rel bias (k==0 in the pad).
            (ScalarE copy with a stride-0 src was tried: device INTERNAL
            error -- keep these on the DVE.)"""
            zt = relc[:, 0:1]
            for c0, c1 in ranges:
                dst = planes[:, 0, :, c0:c1]
                src = bass.AP(
                    zt.tensor, zt.offset, [zt.ap[0], [0, SLAB], [0, c1 - c0]]
                )
                nc.vector.tensor_copy(dst, src)

        def chain(planes, r0=0, r1=SLAB):
            """planes 1..6 = plane0 + (rel[m]-rel[0]), full width incl pads.
            Plane m is only ever read at rows [m, m+24), so clip per m."""
            for m in range(1, K):
                a, b = max(r0, m), min(r1, m + RH)
                if a >= b:
                    continue
                nc.vector.tensor_scalar(
                    out=planes[:, m, a:b], in0=planes[:, 0, a:b],
                    scalar1=relc[:, m : m + 1], scalar2=None,
                    op0=mybir.AluOpType.add,
                )

        strips(planesA, [(0, 3), (99, 102)])
        chain(planesA, 0, 14)  # rows 0..13 gate the first logit

        # ---- main loop: 3 epochs x 7 dC groups, software-pipelined by one
        # group so ScalarE's EXP[g] runs while group g+1's logits build.
        lgp = ctx.enter_context(tc.tile_pool(name="lgp", bufs=2))
        ep_pool = ctx.enter_context(tc.tile_pool(name="ep", bufs=3))
        outp = ctx.enter_context(tc.tile_pool(name="outp", bufs=1))
        ps_loop = ctx.enter_context(tc.tile_pool(name="ps_loop", bufs=2, space="PSUM"))

        nd_tiles = {}

        def phase_logit(e, dC):
            """logit[dR] = q * plane_dR window  (one TT, all 7 dR)."""
            P = dC & 1
            dC2 = dC - P
            planes = planesB if P else planesA
            lg = lgp.tile([128, K, FD], FP16, tag="lg")
            pl = planes[:, 0]
            in1 = bass.AP(
                pl.tensor, pl.offset + EP_ROWS * e * PW + dC2,
                [pl.ap[0], [PLSTRIDE + PW, K], [PW, EP_ROWS], [1, W]],
            )
            qs = q_sbs[e][:, 0:FD]
            q_bc = bass.AP(
                qs.tensor, qs.offset,
                [qs.ap[0], [0, K], [W, EP_ROWS], [1, W]],
            )
            nc.vector.tensor_tensor(
                out=lg.rearrange("p k (r w) -> p k r w", r=EP_ROWS),
                in0=q_bc, in1=in1, op=mybir.AluOpType.mult,
            )
            return lg

        # NOTE: offloading e*v taps to GpSimd was tried and REGRESSED badly:
        # GpSimd shares SBUF ports with the DVE, so concurrent GpSimd tensor
        # ops knock the DVE's tensor_tensor out of 2x mode (~2x slowdown).
        GP_TAPS = 0  # dR taps of the e*v product offloaded to GpSimd

        def phase_rest(e, dC, lg, split=False):
            """exp, e*v, and the accumulate matmuls for group (e, dC). With
            split=True (the very last group) run in two dR-halves so the
            tail matmuls/normalize start earlier."""
            P = dC & 1
            dC2 = dC - P
            vbuf = vbufB if P else vbufA
            ept = ep_pool.tile([128, K, 2, FD], BF16, tag="ept")
            if dC == 0:
                nd_tiles[e] = ps_loop.tile(
                    [128, len(CHUNKS), 512], F32, tag="nd", name=f"nd{e}"
                )
            nd_ps = nd_tiles[e]
            rowstep = vbuf.ap[1][0]

            def prod(i0, i1, eng):
                vbase = vbuf[
                    :, EP_ROWS * e + i0 : EP_ROWS * e + i0 + 1, dC2 : dC2 + W
                ]
                v_ov = bass.AP(
                    vbase.tensor, vbase.offset,
                    [vbase.ap[0], [rowstep, i1 - i0], [rowstep, EP_ROWS], [1, W]],
                )
                eng.tensor_tensor(
                    out=ept[:, i0:i1, 0, :].rearrange(
                        "p k (r w) -> p k r w", r=EP_ROWS
                    ),
                    in0=ept[:, i0:i1, 1, :].rearrange(
                        "p k (r w) -> p k r w", r=EP_ROWS
                    ),
                    in1=v_ov, op=mybir.AluOpType.mult,
                )

            halves = [(0, 4), (4, K)] if split else [(0, K)]
            for h0, h1 in halves:
                nc.scalar.activation(
                    out=ept[:, h0:h1, 1, :], in_=lg[:, h0:h1, :],
                    func=mybir.ActivationFunctionType.Exp, bias=0.0, scale=1.0,
                )
                gp0 = max(h0, K - GP_TAPS)
                if gp0 > h0:
                    prod(h0, min(h1, gp0), nc.vector)
                if h1 > gp0:
                    prod(gp0, h1, nc.gpsimd)
                for i in range(h0, h1):
                    for c, (px0, cw) in enumerate(CHUNKS):
                        nc.tensor.matmul(
                            nd_ps[:, c, 0 : 2 * cw], ident,
                            ept[:, i, :, px0 : px0 + cw],
                            start=(dC == 0 and i == 0),
                            stop=(dC == K - 1 and i == K - 1),
                        )

        def normalize(e):
            """out = num * exp(-ln(den)); [num|den] interleaved 256s. The
            last epoch pipelines ln/recip/mult/DMA per 256-px chunk to
            shorten the serial tail; earlier epochs have slack, so one op
            each keeps ACT/DVE instruction overhead down."""
            nd_ps = nd_tiles[e]
            NC = len(CHUNKS)
            lnden = outp.tile([128, FD], F32, tag="lnden")
            recip = outp.tile([128, FD], F32, tag="recip")
            out_sb = outp.tile([128, FD], F32, tag="out_sb")
            pieces = (
                [(c, c + 1) for c in range(NC)] if e == EP - 1 else [(0, NC)]
            )
            for c0, c1 in pieces:
                w0, w1 = c0 * 256, c1 * 256
                nc.scalar.activation(
                    out=lnden[:, w0:w1].rearrange("p (c w) -> p c w", c=c1 - c0),
                    in_=nd_ps[:, c0:c1, 256:512],
                    func=mybir.ActivationFunctionType.Ln, bias=0.0, scale=1.0,
                )
                nc.scalar.activation(
                    out=recip[:, w0:w1], in_=lnden[:, w0:w1],
                    func=mybir.ActivationFunctionType.Exp, bias=0.0, scale=-1.0,
                )
                nc.vector.tensor_tensor(
                    out=out_sb[:, w0:w1].rearrange("p (c w) -> p c w", c=c1 - c0),
                    in0=nd_ps[:, c0:c1, 0:256],
                    in1=recip[:, w0:w1].rearrange("p (c w) -> p c w", c=c1 - c0),
                    op=mybir.AluOpType.mult,
                )
                nc.sync.dma_start(
                    out=out_ext[:, e * FD + w0 : e * FD + w1],
                    in_=out_sb[:, w0:w1],
                )

        groups = [(e, dC) for e in range(EP) for dC in range(K)]
        pending = None  # (e, dC, lg) whose exp/prod/MMs are not yet emitted
        for e, dC in groups:
            lg = phase_logit(e, dC)
            if (e, dC) == (0, 0):
                # B planes are first needed by group (0,1); building them
                # here overlaps the chain with group (0,0)'s exp.
                strips(planesB, [(0, 2), (98, 102)])
                chain(planesB, 0, 14)
            elif (e, dC) == (0, 1):
                chain(planesA, 14, SLAB)  # needed from epoch 1 on
            elif (e, dC) == (0, 2):
                chain(planesB, 14, SLAB)
            if pending is not None:
                pe, pc, plg = pending
                phase_rest(pe, pc, plg)
                if pc == K - 1:
                    normalize(pe)
            pending = (e, dC, lg)
        pe, pc, plg = pending
        phase_rest(pe, pc, plg)
        normalize(pe)

    _split_sync_waits(nc)
    return nc


def _host_prep(x, v, w_q, w_k, rel_h, rel_w):
    """Build the 8 per-core input maps (numpy only)."""
    x = np.asarray(x, np.float32)
    v = np.asarray(v, np.float32)
    w_q = np.asarray(w_q, np.float32)
    w_k = np.asarray(w_k, np.float32)
    rel_h = np.asarray(rel_h, np.float32).reshape(64, K)   # [c, i]
    rel_w = np.asarray(rel_w, np.float32).reshape(64, K)   # [c, j]

    ident = np.eye(128, dtype=np.float32).astype(ml_dtypes.bfloat16)

    in_maps = []
    for ci in range(NCORES):
        b, rest = divmod(ci, 4)
        half, t = divmod(rest, 2)
        ch0 = 64 * half
        if half == 0:
            xf = x[b]                          # [128, R=h, C=w]
            vf = v[b, ch0 : ch0 + 64]
            relv = rel_h                       # [c, m] with m = dR
        else:
            xf = np.ascontiguousarray(x[b].transpose(0, 2, 1))   # R=w, C=h
            vf = np.ascontiguousarray(v[b, ch0 : ch0 + 64].transpose(0, 2, 1))
            relv = rel_w

        R0 = RT * t
        # x slab: stored rows R0-3 .. R0+50, zero beyond the image
        xs = np.zeros((128, XSLAB, W), np.float32)
        glo, ghi = max(0, R0 - PAD), min(96, R0 + RT + PAD)
        xs[:, glo - (R0 - PAD) : ghi - (R0 - PAD), :] = xf[:, glo:ghi, :]

        # v family buffer: partition p = c + 64*rh
        vs = np.zeros((2, 64, SLAB, WPAD), np.float32)
        for rh in range(2):
            r0 = R0 + RH * rh
            lo, hi = max(0, r0 - PAD), min(96, r0 + RH + PAD)
            vs[rh, :, lo - (r0 - PAD) : hi - (r0 - PAD), PAD : PAD + W] = (
                vf[:, lo:hi, :]
            )
        vbufA = vs.reshape(128, SLAB, WPAD)
        vbufB = np.zeros_like(vbufA)
        vbufB[:, :, : WPAD - 1] = vbufA[:, :, 1:]

        relc = np.zeros((128, 8), np.float32)
        rv = np.concatenate([relv, relv], axis=0)          # [128, 7], p=c+64rh
        relc[:, 0] = rv[:, 0]
        relc[:, 1:K] = rv[:, 1:K] - rv[:, 0:1]

        in_maps.append(
            {
                "xs": np.ascontiguousarray(xs.reshape(128, XSLAB * W)).astype(
                    np.float16
                ),
                "vbufA": np.ascontiguousarray(vbufA.astype(ml_dtypes.bfloat16)),
                "vbufB": np.ascontiguousarray(vbufB.astype(ml_dtypes.bfloat16)),
                "wqT": np.ascontiguousarray(w_q[ch0 : ch0 + 64].T).astype(np.float16),
                "wkT": np.ascontiguousarray(w_k[ch0 : ch0 + 64].T).astype(np.float16),
                "relcols": relc,
                "ident": ident,
            }
        )
    return in_maps


def kernel(x, v, w_q, w_k, rel_h, rel_w, trace=False, tmpdir=None):
    from concourse.bass_utils import run_bass_kernel_spmd

    if "nc" not in _cache:
        _cache["nc"] = _build()
    nc = _cache["nc"]
    in_maps = _host_prep(x, v, w_q, w_k, rel_h, rel_w)
    res = run_bass_kernel_spmd(
        nc, in_maps, list(range(NCORES)), trace=trace, tmpdir=tmpdir
    )
    out = np.zeros((B, COUT, H, W), np.float32)
    for ci in range(NCORES):
        b, rest = divmod(ci, 4)
        half, t = divmod(rest, 2)
        ch0, R0 = 64 * half, RT * t
        a = res.results[ci]["out"].reshape(2, 64, EP, EP_ROWS, W)
        lines = a.transpose(1, 0, 2, 3, 4).reshape(64, RT, W)  # [c, line, C]
        if half == 0:
            out[b, ch0 : ch0 + 64, R0 : R0 + RT, :] = lines
        else:
            out[b, ch0 : ch0 + 64, :, R0 : R0 + RT] = lines.transpose(0, 2, 1)
    kernel.last_exec_time_ns = res.exec_time_ns
    kernel.last_results = res
    return out


# revision 33
# speedup vs baseline: 1.0088x; 1.0088x over previous
"""Trainium2 Bass kernel for 7x7 local (sparse) attention, SPMD over 8 NeuronCores.

Math (per channel c, pixel p):
    q = w_q @ x, k = w_k @ x            (1x1 convs)
    logit[c,p,(i,j)] = q[c,p] * (kpad[c, p+(i,j)] + rel[c,(i,j)])
    out[c,p] = sum_k softmax_k(logit) * vpad[c, p+(i,j)]
where rel[c,(i,j)] = rel_h[c,i] for c<64, rel_w[c-64,j] for c>=64, and
kpad/vpad are zero-padded by 3 (padded taps contribute exp(q*rel) to the
softmax denominator -- matching the reference exactly).

Sharding: 8 cores = 2 batches x 2 CHANNEL halves x 2 spatial 48-line tiles.
High-half cores store all spatial data TRANSPOSED (W-major), so that the rel
bias always attaches to the stored-ROW tap (rel_h[c,i] for low cores, rel_w
[c,j] for high ones) and one NEFF serves all 8 cores.  Within a core the
128 partitions hold (64 channels x 2 row-halves of 24 lines each).

This layout lets the per-tap "krel = k-window + rel" pass (49 taps worth of
adds per pixel) collapse into SEVEN pre-biased k planes built once per core
(plane_m = k + rel[c,m]; window overlap gives the 7x redundancy win), plus a
1-col-shifted B copy of each for the odd column taps (DVE 2x mode needs
4B-aligned windows).  The logit tensor_tensor then reads plane windows
directly:  logit[c,(dR,dC),p] = q[c,p] * plane_dR[c, p + (dR,dC)].

Per-core device pipeline (21 groups = 3 row-epochs x 7 dC):
  DVE: one batched tensor_tensor per group for logits (q broadcast over the
       7 dR via a stride-0 AP reading the 7 planes via a strided window AP)
       and one for e*v (overlapping-row v AP), both fp16/bf16 2x mode.
  ACT: one exp per group (7 dR batched), and the final recip = exp(-ln(den)).
  PE:  q/k 1x1 convs (64-wide weights, written into both partition halves),
       then one identity-matmul per (dR, 256-px chunk) accumulating
       [num | den] jointly into 5 PSUM banks per epoch.
The emission is software-pipelined by one group so ScalarE's exp[g] overlaps
group g+1's logit TT.  Output epoch = 8 lines x 96 cols: [num|den] for 768 px
= 3 PSUM banks, so two epochs' accumulators coexist and the epoch boundary
(normalize of e vs first matmuls of e+1) fully overlaps.

fp16 is used for the whole logit chain (q, planes, logit): bf16's 8-bit
mantissa on |logit|<=60 costs ~4% absmax error; fp16 keeps it ~1%.
e/prod are bf16 (need range: e up to exp(60)).
"""

import sys

import numpy as np
import ml_dtypes

sys.path.insert(0, "/opt/trn_rl_repo")

import concourse.bass as bass  # noqa: E402
import concourse.tile as tile  # noqa: E402
from concourse import mybir  # noqa: E402
from concourse.vector_clock import ScopedClock  # noqa: E402

F32 = mybir.dt.float32
BF16 = mybir.dt.bfloat16
FP16 = mybir.dt.float16

B, CIN, COUT, H, W, K, PAD = 2, 128, 128, 96, 96, 7, 3
NCORES = 8
RT = 48          # stored-R lines per core
RH = 24          # lines per partition half
SLAB = RH + 2 * PAD   # 30 rows incl halo per half
XSLAB = RT + 2 * PAD  # 54 rows of x slab
PW = 102         # padded plane width (3 + 96 + 3)
WPAD = 104       # v buffer width (96 + 6 pad + 2 spare)
EP_ROWS = 8      # lines per PSUM epoch
EP = RH // EP_ROWS    # 3 epochs
FD = EP_ROWS * W      # 768
PLSTRIDE = SLAB * PW  # 3060 elements per plane
# PSUM chunking: each bank holds [num(256 px) | den(same px)] so one matmul
# accumulates both. 768 px = 3 chunks of 256 -> 3 banks, 3 MMs; two epochs'
# accumulators coexist (3+3 of 8 banks) so the epoch boundary overlaps.
CHUNKS = [(0, 256), (256, 256), (512, 256)]

_cache = {}


def _patch_tile_drain():
    """walrus in this container allows only one sync-wait per instruction;
    split excess waits onto NoOps."""
    if getattr(tile, "_drain_patched", False):
        return

    def _drain_and_barrier(self, tick_clock, wait_clock):
        drain_inst = self.nc.sync.drain()
        wait_clock.add_sem_waits(
            drain_inst.ins, ScopedClock({None: tick_clock.global_clock})
        )
        si = drain_inst.ins.sync_info
        if si is not None and si.on_wait and len(si.on_wait) > 1:
            waits = list(si.on_wait)
            drain_inst.ins.sync_info = mybir.SyncInfo(
                on_wait=waits[:1], on_update=list(si.on_update)
            )
            for w in waits[1:]:
                nop_inst = self.nc.sync.nop()
                nop_inst.ins.sync_info = mybir.SyncInfo(on_wait=[w], on_update=[])
        self.nc.all_engine_barrier()
        assert self.sems is not None
        popped = self.nc._tile_sem_poison_stack.pop()
        assert popped is self._sem_poison
        self.nc.clear_and_free_semaphores(list(self.sems.allocated().values()))
        self.nc.all_engine_barrier()

    tile.TileContext._drain_and_barrier = _drain_and_barrier
    tile._drain_patched = True


_split_ctr = [0]


def _split_sync_waits(nc, maxw=1):
    for fn in nc.m.functions:
        for bb in fn.blocks:
            if not any(
                inst.sync_info is not None
                and inst.sync_info.on_wait
                and len(inst.sync_info.on_wait) > maxw
                for inst in bb.instructions
            ):
                continue
            new_list = []
            for inst in bb.instructions:
                si = inst.sync_info
                if si is not None and si.on_wait and len(si.on_wait) > maxw:
                    waits = list(si.on_wait)
                    extra, keep = waits[:-maxw], waits[-maxw:]
                    for i in range(0, len(extra), maxw):
                        _split_ctr[0] += 1
                        nop = mybir.InstNoOp(
                            name=f"splitw-{_split_ctr[0]}", ins=[], outs=[]
                        )
                        nop.engine = inst.engine
                        nop.sync_info = mybir.SyncInfo(
                            on_wait=extra[i : i + maxw], on_update=[]
                        )
                        new_list.append(nop)
                    inst.sync_info = mybir.SyncInfo(
                        on_wait=keep, on_update=list(si.on_update)
                    )
                new_list.append(inst)
            try:
                bb.instructions = new_list
            except Exception:
                bb.instructions.clear()
                bb.instructions.extend(new_list)


def _build():
    _patch_tile_drain()
    nc = bass.Bass("TRN2", target_bir_lowering=False, debug=False)

    xs_ext = nc.dram_tensor("xs", [128, XSLAB * W], FP16, kind="ExternalInput").ap()
    va_ext = nc.dram_tensor("vbufA", [128, SLAB, WPAD], BF16, kind="ExternalInput").ap()
    vb_ext = nc.dram_tensor("vbufB", [128, SLAB, WPAD], BF16, kind="ExternalInput").ap()
    wq_ext = nc.dram_tensor("wqT", [128, 64], FP16, kind="ExternalInput").ap()
    wk_ext = nc.dram_tensor("wkT", [128, 64], FP16, kind="ExternalInput").ap()
    rel_ext = nc.dram_tensor("relcols", [128, 8], F32, kind="ExternalInput").ap()
    id_ext = nc.dram_tensor("ident", [128, 128], BF16, kind="ExternalInput").ap()
    out_ext = nc.dram_tensor("out", [128, RH * W], F32, kind="ExternalOutput").ap()

    from contextlib import ExitStack

    with tile.TileContext(nc) as tc, ExitStack() as ctx:
        consts = ctx.enter_context(tc.tile_pool(name="consts", bufs=1))
        main = ctx.enter_context(tc.tile_pool(name="main", bufs=1))

        wqT = consts.tile([128, 64], FP16)
        wkT = consts.tile([128, 64], FP16)
        ident = consts.tile([128, 128], BF16)
        relc = consts.tile([128, 8], F32)
        vbufA = main.tile([128, SLAB, WPAD], BF16)
        vbufB = main.tile([128, SLAB, WPAD], BF16)
        q_sbs = [main.tile([128, FD], FP16, name=f"q{e}") for e in range(EP)]
        planesA = main.tile([128, K, SLAB, PW], FP16)
        planesB = main.tile([128, K, SLAB, PW], FP16)

        # ---- preamble: q/k 1x1 convs into the (channel, row-half) layout
        with tc.tile_pool(name="xsp", bufs=1) as xsp:
            xs = xsp.tile([128, XSLAB * W], FP16)
            # xs alone on the sync queue (it gates the k matmuls); weights +
            # bulk v + consts on scalar so they land in parallel.
            nc.sync.dma_start(out=xs[:, 0 : SLAB * W], in_=xs_ext[:, 0 : SLAB * W])
            nc.sync.dma_start(
                out=xs[:, SLAB * W : XSLAB * W], in_=xs_ext[:, SLAB * W : XSLAB * W]
            )
            nc.scalar.dma_start(out=wkT, in_=wk_ext)
            nc.scalar.dma_start(out=wqT, in_=wq_ext)
            nc.scalar.dma_start(out=relc, in_=rel_ext)
            nc.scalar.dma_start(out=ident, in_=id_ext)
            nc.scalar.dma_start(out=vbufA, in_=va_ext)
            nc.scalar.dma_start(out=vbufB, in_=vb_ext)
            # dummy first ACTIVATE: hoists the ~1.3us ACT_TABLE_LOAD off the
            # k_conv -> plane0 critical path (it attaches to the first ACT op)
            scr = consts.tile([128, 1], F32)
            nc.scalar.activation(
                out=scr, in_=relc[:, 0:1],
                func=mybir.ActivationFunctionType.Exp, bias=0.0, scale=1.0,
            )

            NPRE = 6
            KW = SLAB * W // NPRE  # 480 = 5 rows
            QW = RH * W // NPRE    # 384 = 4 rows
            with tc.tile_pool(name="ps_pre", bufs=1, space="PSUM") as ps_pre:
                # k first: it gates the plane build (q only gates logits)
                k_ps = ps_pre.tile([128, NPRE, 512], F32, tag="pre")
                # chunk-major: plane0A's first half reads chunks 0-2 of BOTH
                # partition halves, so emit those 6 matmuls first
                for c in range(NPRE):
                    for rh in range(2):
                        base = RH * rh * W
                        nc.tensor.matmul(
                            k_ps[64 * rh : 64 * rh + 64, c, 0:KW],
                            wkT,
                            xs[:, base + c * KW : base + (c + 1) * KW],
                            start=True, stop=True,
                        )
                # plane 0 (A and B copies) straight from PSUM with the m=0
                # rel bias; chunk c covers slab rows 5c..5c+4
                k_src = k_ps[:, :, 0:KW].rearrange("p c (r w) -> p c r w", r=5)
                p0A = planesA[:, 0, :, 3 : 3 + W].rearrange(
                    "p (c r) w -> p c r w", c=NPRE
                )
                p0B = planesB[:, 0, :, 2 : 2 + W].rearrange(
                    "p (c r) w -> p c r w", c=NPRE
                )
                # plane0A in two row-halves: the first logit only reads plane
                # rows 0..13, so it can start before the bottom half lands.
                nc.scalar.activation(
                    out=p0A[:, 0:3], in_=k_src[:, 0:3],
                    func=mybir.ActivationFunctionType.Identity,
                    bias=relc[:, 0:1], scale=1.0,
                )
                nc.scalar.activation(
                    out=p0A[:, 3:NPRE], in_=k_src[:, 3:NPRE],
                    func=mybir.ActivationFunctionType.Identity,
                    bias=relc[:, 0:1], scale=1.0,
                )
                # q gets its OWN 2 PSUM banks (6 k + 2 q = 8): reusing the k
                # banks would stall the q convs behind the plane0 ACT reads.
                # Both partition halves share a bank; one wave per epoch.
                q_ps = ps_pre.tile([128, 2, 512], F32, tag="qps")
                for w in range(EP):
                    for j in range(2):
                        for rh in range(2):
                            base = (RH * rh + PAD) * W + (2 * w + j) * QW
                            nc.tensor.matmul(
                                q_ps[64 * rh : 64 * rh + 64, j, 0:QW],
                                wqT,
                                xs[:, base : base + QW],
                                start=True, stop=True,
                            )
                    nc.scalar.copy(
                        out=q_sbs[w].rearrange("p (c w) -> p c w", c=2),
                        in_=q_ps[:, :, 0:QW],
                    )
                    if w == 0:
                        nc.scalar.activation(
                            out=p0B, in_=k_src,
                            func=mybir.ActivationFunctionType.Identity,
                            bias=relc[:, 0:1], scale=1.0,
                        )

        def strips(planes, ranges):
            """pad strips for plane 0: value = rel bias (k==0 in the pad).
            (ScalarE copy with a stride-0 src was tried: device INTERNAL
            error -- keep these on the DVE.)"""
            zt = relc[:, 0:1]
            for c0, c1 in ranges:
                dst = planes[:, 0, :, c0:c1]
                src = bass.AP(
                    zt.tensor, zt.offset, [zt.ap[0], [0, SLAB], [0, c1 - c0]]
                )
                nc.vector.tensor_copy(dst, src)

        def chain(planes, r0=0, r1=SLAB):
            """planes 1..6 = plane0 + (rel[m]-rel[0]), full width incl pads.
            Plane m is only ever read at rows [m, m+24), so clip per m."""
            for m in range(1, K):
                a, b = max(r0, m), min(r1, m + RH)
                if a >= b:
                    continue
                nc.vector.tensor_scalar(
                    out=planes[:, m, a:b], in0=planes[:, 0, a:b],
                    scalar1=relc[:, m : m + 1], scalar2=None,
                    op0=mybir.AluOpType.add,
                )

        strips(planesA, [(0, 3), (99, 102)])
        chain(planesA, 0, 14)  # rows 0..13 gate the first logit

        # ---- main loop: 3 epochs x 7 dC groups, software-pipelined by one
        # group so ScalarE's EXP[g] runs while group g+1's logits build.
        lgp = ctx.enter_context(tc.tile_pool(name="lgp", bufs=2))
        ep_pool = ctx.enter_context(tc.tile_pool(name="ep", bufs=3))
        outp = ctx.enter_context(tc.tile_pool(name="outp", bufs=1))
        ps_loop = ctx.enter_context(tc.tile_pool(name="ps_loop", bufs=2, space="PSUM"))

        nd_tiles = {}

        def phase_logit(e, dC):
            """logit[dR] = q * plane_dR window  (one TT, all 7 dR)."""
            P = dC & 1
            dC2 = dC - P
            planes = planesB if P else planesA
            lg = lgp.tile([128, K, FD], FP16, tag="lg")
            pl = planes[:, 0]
            in1 = bass.AP(
                pl.tensor, pl.offset + EP_ROWS * e * PW + dC2,
                [pl.ap[0], [PLSTRIDE + PW, K], [PW, EP_ROWS], [1, W]],
            )
            qs = q_sbs[e][:, 0:FD]
            q_bc = bass.AP(
                qs.tensor, qs.offset,
                [qs.ap[0], [0, K], [W, EP_ROWS], [1, W]],
            )
            nc.vector.tensor_tensor(
                out=lg.rearrange("p k (r w) -> p k r w", r=EP_ROWS),
                in0=q_bc, in1=in1, op=mybir.AluOpType.mult,
            )
            return lg

        # NOTE: offloading e*v taps to GpSimd was tried and REGRESSED badly:
        # GpSimd shares SBUF ports with the DVE, so concurrent GpSimd tensor
        # ops knock the DVE's tensor_tensor out of 2x mode (~2x slowdown).
        GP_TAPS = 0  # dR taps of the e*v product offloaded to GpSimd

        def phase_rest(e, dC, lg, split=False):
            """exp, e*v, and the accumulate matmuls for group (e, dC). With
            split=True (the very last group) run in two dR-halves so the
            tail matmuls/normalize start earlier."""
            P = dC & 1
            dC2 = dC - P
            vbuf = vbufB if P else vbufA
            ept = ep_pool.tile([128, K, 2, FD], BF16, tag="ept")
            if dC == 0:
                nd_tiles[e] = ps_loop.tile(
                    [128, len(CHUNKS), 512], F32, tag="nd", name=f"nd{e}"
                )
            nd_ps = nd_tiles[e]
            rowstep = vbuf.ap[1][0]

            def prod(i0, i1, eng):
                vbase = vbuf[
                    :, EP_ROWS * e + i0 : EP_ROWS * e + i0 + 1, dC2 : dC2 + W
                ]
                v_ov = bass.AP(
                    vbase.tensor, vbase.offset,
                    [vbase.ap[0], [rowstep, i1 - i0], [rowstep, EP_ROWS], [1, W]],
                )
                eng.tensor_tensor(
                    out=ept[:, i0:i1, 0, :].rearrange(
                        "p k (r w) -> p k r w", r=EP_ROWS
                    ),
                    in0=ept[:, i0:i1, 1, :].rearrange(
                        "p k (r w) -> p k r w", r=EP_ROWS
                    ),
                    in1=v_ov, op=mybir.AluOpType.mult,
                )

            halves = [(0, 4), (4, K)] if split else [(0, K)]
            for h0, h1 in halves:
                nc.scalar.activation(
                    out=ept[:, h0:h1, 1, :], in_=lg[:, h0:h1, :],
                    func=mybir.ActivationFunctionType.Exp, bias=0.0, scale=1.0,
                )
                gp0 = max(h0, K - GP_TAPS)
                if gp0 > h0:
                    prod(h0, min(h1, gp0), nc.vector)
                if h1 > gp0:
                    prod(gp0, h1, nc.gpsimd)
                for i in range(h0, h1):
                    for c, (px0, cw) in enumerate(CHUNKS):
                        nc.tensor.matmul(
                            nd_ps[:, c, 0 : 2 * cw], ident,
                            ept[:, i, :, px0 : px0 + cw],
                            start=(dC == 0 and i == 0),
                            stop=(dC == K - 1 and i == K - 1),
                        )

        def normalize(e):
            """out = num * exp(-ln(den)); [num|den] interleaved 256s."""
            nd_ps = nd_tiles[e]
            NC = len(CHUNKS)
            lnden = outp.tile([128, FD], F32, tag="lnden")
            nc.scalar.activation(
                out=lnden.rearrange("p (c w) -> p c w", c=NC),
                in_=nd_ps[:, :, 256:512],
                func=mybir.ActivationFunctionType.Ln, bias=0.0, scale=1.0,
            )
            recip = outp.tile([128, FD], F32, tag="recip")
            nc.scalar.activation(
                out=recip, in_=lnden,
                func=mybir.ActivationFunctionType.Exp, bias=0.0, scale=-1.0,
            )
            out_sb = outp.tile([128, FD], F32, tag="out_sb")
            nc.vector.tensor_tensor(
                out=out_sb.rearrange("p (c w) -> p c w", c=NC),
                in0=nd_ps[:, :, 0:256],
                in1=recip.rearrange("p (c w) -> p c w", c=NC),
                op=mybir.AluOpType.mult,
            )
            nc.sync.dma_start(out=out_ext[:, e * FD : (e + 1) * FD], in_=out_sb)

        groups = [(e, dC) for e in range(EP) for dC in range(K)]
        pending = None  # (e, dC, lg) whose exp/prod/MMs are not yet emitted
        for e, dC in groups:
            lg = phase_logit(e, dC)
            if (e, dC) == (0, 0):
                # B planes are first needed by group (0,1); building them
                # here overlaps the chain with group (0,0)'s exp.
                strips(planesB, [(0, 2), (98, 102)])
                chain(planesB, 0, 14)
            elif (e, dC) == (0, 1):
                chain(planesA, 14, SLAB)  # needed from epoch 1 on
            elif (e, dC) == (0, 2):
                chain(planesB, 14, SLAB)
            if pending is not None:
                pe, pc, plg = pending
                phase_rest(pe, pc, plg)
                if pc == K - 1:
                    normalize(pe)
            pending = (e, dC, lg)
        pe, pc, plg = pending
        phase_rest(pe, pc, plg)
        normalize(pe)

    _split_sync_waits(nc)
    return nc


def _host_prep(x, v, w_q, w_k, rel_h, rel_w):
    """Build the 8 per-core input maps (numpy only)."""
    x = np.asarray(x, np.float32)
    v = np.asarray(v, np.float32)
    w_q = np.asarray(w_q, np.float32)
    w_k = np.asarray(w_k, np.float32)
    rel_h = np.asarray(rel_h, np.float32).reshape(64, K)   # [c, i]
    rel_w = np.asarray(rel_w, np.float32).reshape(64, K)   # [c, j]

    ident = np.eye(128, dtype=np.float32).astype(ml_dtypes.bfloat16)

    in_maps = []
    for ci in range(NCORES):
        b, rest = divmod(ci, 4)
        half, t = divmod(rest, 2)
        ch0 = 64 * half
        if half == 0:
            xf = x[b]                          # [128, R=h, C=w]
            vf = v[b, ch0 : ch0 + 64]
            relv = rel_h                       # [c, m] with m = dR
        else:
            xf = np.ascontiguousarray(x[b].transpose(0, 2, 1))   # R=w, C=h
            vf = np.ascontiguousarray(v[b, ch0 : ch0 + 64].transpose(0, 2, 1))
            relv = rel_w

        R0 = RT * t
        # x slab: stored rows R0-3 .. R0+50, zero beyond the image
        xs = np.zeros((128, XSLAB, W), np.float32)
        glo, ghi = max(0, R0 - PAD), min(96, R0 + RT + PAD)
        xs[:, glo - (R0 - PAD) : ghi - (R0 - PAD), :] = xf[:, glo:ghi, :]

        # v family buffer: partition p = c + 64*rh
        vs = np.zeros((2, 64, SLAB, WPAD), np.float32)
        for rh in range(2):
            r0 = R0 + RH * rh
            lo, hi = max(0, r0 - PAD), min(96, r0 + RH + PAD)
            vs[rh, :, lo - (r0 - PAD) : hi - (r0 - PAD), PAD : PAD + W] = (
                vf[:, lo:hi, :]
            )
        vbufA = vs.reshape(128, SLAB, WPAD)
        vbufB = np.zeros_like(vbufA)
        vbufB[:, :, : WPAD - 1] = vbufA[:, :, 1:]

        relc = np.zeros((128, 8), np.float32)
        rv = np.concatenate([relv, relv], axis=0)          # [128, 7], p=c+64rh
        relc[:, 0] = rv[:, 0]
        relc[:, 1:K] = rv[:, 1:K] - rv[:, 0:1]

        in_maps.append(
            {
                "xs": np.ascontiguousarray(xs.reshape(128, XSLAB * W)).astype(
                    np.float16
                ),
                "vbufA": np.ascontiguousarray(vbufA.astype(ml_dtypes.bfloat16)),
                "vbufB": np.ascontiguousarray(vbufB.astype(ml_dtypes.bfloat16)),
                "wqT": np.ascontiguousarray(w_q[ch0 : ch0 + 64].T).astype(np.float16),
                "wkT": np.ascontiguousarray(w_k[ch0 : ch0 + 64].T).astype(np.float16),
                "relcols": relc,
                "ident": ident,
            }
        )
    return in_maps


def kernel(x, v, w_q, w_k, rel_h, rel_w, trace=False, tmpdir=None):
    from concourse.bass_utils import run_bass_kernel_spmd

    if "nc" not in _cache:
        _cache["nc"] = _build()
    nc = _cache["nc"]
    in_maps = _host_prep(x, v, w_q, w_k, rel_h, rel_w)
    res = run_bass_kernel_spmd(
        nc, in_maps, list(range(NCORES)), trace=trace, tmpdir=tmpdir
    )
    out = np.zeros((B, COUT, H, W), np.float32)
    for ci in range(NCORES):
        b, rest = divmod(ci, 4)
        half, t = divmod(rest, 2)
        ch0, R0 = 64 * half, RT * t
        a = res.results[ci]["out"].reshape(2, 64, EP, EP_ROWS, W)
        lines = a.transpose(1, 0, 2, 3, 4).reshape(64, RT, W)  # [c, line, C]
        if half == 0:
            out[b, ch0 : ch0 + 64, R0 : R0 + RT, :] = lines
        else:
            out[b, ch0 : ch0 + 64, :, R0 : R0 + RT] = lines.transpose(0, 2, 1)
    kernel.last_exec_time_ns = res.exec_time_ns
    kernel.last_results = res
    return out
